# revision 1
# baseline (speedup 1.0000x reference)
"""Bilateral blur (7x7, L1 color distance) on 8 Trainium2 NeuronCores.

Input (4, 3, 512, 512) fp32 -> output (4, 3, 512, 512) fp32.

Sharding: pure data parallelism - core i handles batch i//2, row-half i%2
(256 output rows x 512 cols). The host reflect-pads each image and re-lays
each shard into a "transposed-blocked" layout: partition p (0..127) owns
output columns [4p, 4p+4); its free dim holds, per channel, a 10x262 patch
(padded cols 4p..4p+9 x 262 padded rows, flattened xl*262+y). TRN2 compute
engines cannot read across partitions, so this layout turns all 49 tap
shifts (dy, dx) into pure free-dim AP offsets (dx*262 + dy).

Per tap: dlt = taps-center (fat fp16 TT over 3 channels), |dlt| (ACT Abs),
d = ch-sum (2 TT), q = d^2 (ACT Square), w = exp(-50q + ln s_k) (ACT Exp,
scale/bias immediates), products w*tap (fp16 TT), and a single fat fp32
accumulate of [m0|m1|m2|w]. Final out = num * reciprocal(den).

fp16 notes: all hot DVE ops are TensorTensor (2x DVE mode in
fp16; the TensorScalarPtr family is 1x-only on this ISA so it is avoided).
-50 rides the exp's scale immediate; ln(s_k) rides its per-partition bias AP
(from a small DMA'd table). Taps are read from one of two host-provided fp16
copies (second shifted one row) so every tap AP is 4-byte aligned (dy parity),
which the DVE 2x mode requires.
"""
import numpy as np

import concourse.bass as bass
import concourse.bacc as bacc
import concourse.mybir as mybir
from concourse.tile import TileContext
from concourse import bass_utils

C = 3
B, H, W = 4, 512, 512
KX = KY = 7
PAD = 3
SIGMA_COLOR = 0.1
N_CORES = 8

ROWS = 256
WG = 4
NPART = 128
XE, YE = WG + 2 * PAD, ROWS + 2 * PAD
FREE_IN = XE * YE
FREE_OUT = WG * ROWS
NTAP = KY * KX


def _space_kernel():
    def g1(k, sigma):
        x = np.arange(k, dtype=np.float64) - (k - 1) / 2.0
        g = np.exp(-0.5 * (x / sigma) ** 2)
        return g / g.sum()
    gy, gx = g1(KY, 1.5), g1(KX, 1.5)
    return (gy[:, None] * gx[None, :]).reshape(-1)


def _tap_ap(t, dx, dy, nch=C, ch0=0, dy_base=0):
    a = t[:]
    return bass.AP(a.tensor, a.offset + ch0 * FREE_IN + dx * YE + (dy - dy_base),
                   [[C * FREE_IN, NPART], [FREE_IN, nch], [YE, WG], [1, ROWS]])


def _stk_ap(t, nch=C, ch0=0, step0=False):
    a = t[:]
    tot = a.shape[1]
    return bass.AP(a.tensor, a.offset + ch0 * FREE_OUT,
                   [[tot, NPART], [0 if step0 else FREE_OUT, nch], [ROWS, WG], [1, ROWS]])


def _build(ntaps=NTAP):
    nc = bacc.Bacc()
    f32 = mybir.dt.float32
    f16 = mybir.dt.float16
    xe = nc.dram_tensor("xe", [NPART, C * FREE_IN], f16, kind="ExternalInput")
    xo = nc.dram_tensor("xo", [NPART, C * FREE_IN], f16, kind="ExternalInput")
    lnsb = nc.dram_tensor("lnsb", [NPART, NTAP], f32, kind="ExternalInput")
    ident = nc.dram_tensor("ident", [NPART, NPART], f16, kind="ExternalInput")
    o = nc.dram_tensor("o", [NPART, C * FREE_OUT], f32, kind="ExternalOutput")
    AOT = mybir.AluOpType
    F = FREE_OUT
    SCL = -0.5 / SIGMA_COLOR ** 2

    with TileContext(nc) as tc:
        with tc.tile_pool(name="persist", bufs=1) as pool, \
             tc.tile_pool(name="tmp", bufs=6) as tp, \
             tc.tile_pool(name="ps", bufs=1, space="PSUM") as psp:
            Te = pool.tile([NPART, C * FREE_IN], f16, name="Te")
            nc.sync.dma_start(Te[:], xe[:])
            To = pool.tile([NPART, C * FREE_IN], f16, name="To")
            nc.sync.dma_start(To[:], xo[:])
            bias = pool.tile([NPART, NTAP], f32, name="bias")
            nc.sync.dma_start(bias[:], lnsb[:])
            idt = pool.tile([NPART, NPART], f16, name="idt")
            nc.sync.dma_start(idt[:], ident[:])

            def tile_for(dy):
                return (Te, 0) if dy % 2 == 0 else (To, 1)

            # fp32 accumulator [m0|m1|m2|w] lives in PSUM: per tap, identity
            # matmuls stream mAll through the PE array, whose PSUM writes
            # accumulate natively in fp32 (exact for fp16 inputs). This takes
            # the entire fp32 accumulation off the DVE critical path.
            acc = psp.tile([NPART, 4 * F], f32, name="acc")
            qb = pool.tile([NPART, F], f16, name="qb")
            MN = 512                       # matmul free-dim per PSUM bank

            tc_t, tc_b = tile_for(PAD)
            ctr = _tap_ap(tc_t, PAD, PAD, dy_base=tc_b)
            for dy in range(KY):
                for dx in range(KX):
                    k = dy * KX + dx
                    if k >= ntaps:
                        continue
                    tt, tb = tile_for(dy)
                    dlt = tp.tile([NPART, C * F], f16, name="dlt", tag="dlt")
                    nc.vector.tensor_tensor(out=_stk_ap(dlt),
                                            in0=_tap_ap(tt, dx, dy, dy_base=tb),
                                            in1=ctr, op=AOT.subtract)
                    adl = tp.tile([NPART, C * F], f16, name="adl", tag="adl")
                    # |.| split across engines: channels 0-1 on ACT, channel 2
                    # on DVE as a sign-bit mask over packed fp16 (u32 view)
                    nc.scalar.activation(adl[:, 0:2 * F], dlt[:, 0:2 * F],
                                         mybir.ActivationFunctionType.Abs,
                                         bias=0.0, scale=1.0)
                    nc.vector.tensor_scalar(
                        out=adl[:, 2 * F:].bitcast(mybir.dt.uint32),
                        in0=dlt[:, 2 * F:].bitcast(mybir.dt.uint32),
                        scalar1=0x7FFF7FFF, scalar2=None,
                        op0=AOT.bitwise_and)
                    # channel-sum on GPSIMD: takes ~1.3K cyc/tap off the DVE
                    # critical path; GPSIMD has slack and its SBUF-port draw
                    # is ~1% of the shared port's bandwidth
                    dsum = tp.tile([NPART, F], f16, name="dsum", tag="dsum")
                    nc.gpsimd.tensor_tensor(out=dsum[:], in0=adl[:, 0:F],
                                            in1=adl[:, F:2 * F], op=AOT.add)
                    nc.gpsimd.tensor_tensor(out=dsum[:], in0=dsum[:],
                                            in1=adl[:, 2 * F:], op=AOT.add)
                    # q = d^2 on ACT (same table set as Exp/Abs -> no set switch);
                    # keeps the DVE critical path shorter
                    nc.scalar.activation(qb[:], dsum[:],
                                         mybir.ActivationFunctionType.Square,
                                         bias=0.0, scale=1.0)
                    mAll = tp.tile([NPART, 4 * F], f16, name="mAll", tag="mAll")
                    # w = exp(-50*q + ln s_k) lands in the 4th slot
                    nc.scalar.activation(mAll[:, 3 * F:], qb[:],
                                         mybir.ActivationFunctionType.Exp,
                                         bias=bias[:, k:k + 1], scale=SCL)
                    wv = mAll[:, 3 * F:]
                    w_b3 = bass.AP(wv.tensor, wv.offset,
                                   [[4 * F, NPART], [0, 3], [ROWS, WG], [1, ROWS]])
                    nc.vector.tensor_tensor(out=_stk_ap(mAll, nch=3), in0=w_b3,
                                            in1=_tap_ap(tt, dx, dy, nch=3, dy_base=tb),
                                            op=AOT.mult)
                    # accumulate on the PE: 8 identity matmuls (one per bank)
                    for g in range(4 * F // MN):
                        nc.tensor.matmul(acc[:, g * MN:(g + 1) * MN], idt[:],
                                         mAll[:, g * MN:(g + 1) * MN],
                                         start=(k == 0), stop=(k == ntaps - 1))

            accm = pool.tile([NPART, 4 * F], f32, name="accm")
            nc.vector.tensor_copy(accm[:], acc[:])
            recip = pool.tile([NPART, F], f32, name="recip")
            nc.vector.reciprocal(recip[:], accm[:, 3 * F:])
            ot = pool.tile([NPART, C * F], f32, name="ot")
            nc.vector.tensor_tensor(out=_stk_ap(ot), in0=accm[:, 0:3 * F],
                                    in1=_stk_ap(recip, step0=True), op=AOT.mult)
            nc.sync.dma_start(o[:], ot[:])
    return nc


_COLIDX = np.arange(NPART)[:, None] * WG + np.arange(XE)[None, :]


def _shard_layout(shard, yshift):
    buf = np.zeros((NPART, C, XE, YE), np.float16)
    for c in range(C):
        blk = shard[c].T[_COLIDX]
        if yshift:
            buf[:, c, :, :YE - yshift] = blk[:, :, yshift:]
        else:
            buf[:, c] = blk
    return buf.reshape(NPART, C * FREE_IN)


_LNSB = np.broadcast_to(
    np.log(_space_kernel()).astype(np.float32)[None, :], (NPART, NTAP)).copy()

_NC_CACHE = {}


def _get_nc():
    if "nc" not in _NC_CACHE:
        nc = _build()
        nc.finalize()
        _NC_CACHE["nc"] = nc
    return _NC_CACHE["nc"]


def make_in_maps(x):
    xp = np.pad(x, ((0, 0), (0, 0), (PAD, PAD), (PAD, PAD)), mode="reflect")
    in_maps = []
    for core in range(N_CORES):
        b, half = core // 2, core % 2
        r0 = half * ROWS
        shard = xp[b, :, r0:r0 + ROWS + 2 * PAD, :]
        in_maps.append({"xe": _shard_layout(shard, 0),
                        "xo": _shard_layout(shard, 1),
                        "lnsb": _LNSB,
                        "ident": np.eye(NPART, dtype=np.float16)})
    return in_maps


def kernel(input: np.ndarray) -> np.ndarray:
    x = np.asarray(input, dtype=np.float32)
    assert x.shape == (B, C, H, W)
    in_maps = make_in_maps(x)
    nc = _get_nc()
    res = bass_utils.run_bass_kernel_spmd(nc, in_maps, list(range(N_CORES)))
    out = np.empty((B, C, H, W), np.float32)
    for core in range(N_CORES):
        b, half = core // 2, core % 2
        r0 = half * ROWS
        ov = np.asarray(res.results[core]["o"]).reshape(NPART, C, WG, ROWS)
        for c in range(C):
            out[b, c, r0:r0 + ROWS, :] = ov[:, c].transpose(2, 0, 1).reshape(ROWS, W)
    return out



# revision 2
# speedup vs baseline: 1.9031x; 1.9031x over previous
"""Bilateral blur (7x7, L1 color distance) on 8 Trainium2 NeuronCores, v2.

Input (4, 3, 512, 512) fp32 -> output (4, 3, 512, 512) fp32.

Sharding: core i handles batch i//2, row-half i%2 (256x512 px). Each of the
128 partitions owns a 32x32 output tile (16 col-groups x 8 row-slices) and
holds the matching 38x38 padded patch per channel in fp16 ([c][xl][yl],
yl contiguous), plus a 1-element-shifted copy so every tap read is 4-byte
aligned (DVE 2x fp16 mode).

Algorithm (per pair of symmetric taps +/-D, D=(dy,dx), 24 pairs + center):
  out = x + (sum_k w_k * dlt_k) / (sum_k w_k),  dlt_k = x(p+D_k) - x(p)
The weight field W(q) = s_D * exp(-50 * d(q)^2), d = sum_c |dlt_c|, is
SYMMETRIC: tap -D at pixel p uses W(p-D), and its numerator contribution is
-P(p-D) where P = W (*) dlt is the same product field used by tap +D. So
distance, exp and the multiply are computed ONCE per pair over a slightly
extended domain (EX=32+|dx|, EY=32+dy), and the PE accumulates both taps
via identity matmuls into PSUM: +I on [P|W](p-domain), -I on P and +I on W
at the mirrored offset. The center tap is exact: contributes only s0 to the
denominator (folded into the PSUM->SBUF copy as an ACT bias).

Engine split per pair (balanced ~4.7us/pair across engines): DVE does the
subtract, |dlt2| (u32 sign-mask at 4 fp16/cyc), the second channel-sum add,
and the product (fp16 TT 2x); ACT does |dlt0|,|dlt1| (one 2-channel Abs),
Square and Exp (s_k rides the exp bias as ln s_k); Pool (GPSIMD) does the
first channel-sum add; PE does 16 accumulate matmuls. Emission is
software-pipelined in 3 stages so each in-order engine queue always has
ready work. ACT tables are pre-warmed during the input DMA.
"""
import numpy as np

import concourse.bass as bass
import concourse.bacc as bacc
import concourse.mybir as mybir
from concourse.tile import TileContext
from concourse import bass_utils

C = 3
B, H, W = 4, 512, 512
PAD = 3
SIGMA_COLOR = 0.1
N_CORES = 8

TS = 32                      # tile side (output px per partition: TS x TS)
PS = TS + 2 * PAD            # padded patch side = 38
NPART = 128
GX, RY = 16, 8               # col-groups x row-slices = 128 partitions
ROWS = RY * TS               # 256 output rows per core
CS = PS * PS                 # per-channel patch stride = 1444
FIN = C * CS                 # 4332
SX = 36                      # field xl stride (even, >= max EY=35)
CF = SX * 35                 # per-channel field stride = 1260 (even)
F = TS * TS                  # 1024
SCL = -0.5 / SIGMA_COLOR ** 2

# symmetric tap pairs: (dy, dx) with dy>0, or dy==0 and dx>0
PAIRS = [(0, dx) for dx in range(1, 4)] + \
        [(dy, dx) for dy in range(1, 4) for dx in range(-3, 4)]
# per-pair engine assignment knobs (k -> bool), tuned via TimelineSim sweeps
CH1_ACT = lambda k: True      # |dlt1| on ACT (with ch0) vs DVE u32
DADD_DVE = lambda k: True     # d = s1+|dlt2| on DVE vs GPSIMD
SUB2_POOL = lambda k: False   # ch2 of the subtract on GPSIMD (parallel lane)
SQ_DVE = lambda k: False      # square on DVE (d*d) vs ACT
ABS3_ACT = lambda k: False    # all 3 abs channels on ACT (no DVE u32 op)
GAUSS_DE = lambda k: False    # Derivative_Erf gaussian + DVE 4x s_k scale
HEAD_OPT = True               # small-table DMAs off the sync queue
DEPTH4 = False                # 4-deep software pipeline (split stage2)
TAIL_OPT = True               # PSUM->SBUF copies before the reciprocal


def _g1(k, sigma):
    x = np.arange(k, dtype=np.float64) - (k - 1) / 2.0
    g = np.exp(-0.5 * (x / sigma) ** 2)
    return g / g.sum()


_G = _g1(7, 1.5)
_S_PAIR = np.array([_G[3 + dy] * _G[3 + dx] for (dy, dx) in PAIRS])
_S0 = float(_G[3] * _G[3])


def _build(ntaps=len(PAIRS)):
    nc = bacc.Bacc()
    f32 = mybir.dt.float32
    f16 = mybir.dt.float16
    xt = nc.dram_tensor("xt", [NPART, FIN], f16, kind="ExternalInput")
    lnsb = nc.dram_tensor("lnsb", [NPART, len(PAIRS) + 1], f32,
                          kind="ExternalInput")
    skt = nc.dram_tensor("skt", [NPART, len(PAIRS)], f32,
                         kind="ExternalInput")
    identp = nc.dram_tensor("identp", [NPART, NPART], f16, kind="ExternalInput")
    identn = nc.dram_tensor("identn", [NPART, NPART], f16, kind="ExternalInput")
    o = nc.dram_tensor("o", [NPART, C * F], f16, kind="ExternalOutput")
    AOT = mybir.AluOpType
    AFT = mybir.ActivationFunctionType

    with TileContext(nc) as tc:
        with tc.tile_pool(name="persist", bufs=1) as pool, \
             tc.tile_pool(name="tmp", bufs=4) as tp, \
             tc.tile_pool(name="ps", bufs=1, space="PSUM") as psp:
            bias = pool.tile([NPART, len(PAIRS) + 1], f32, name="bias")
            nc.sync.dma_start(bias[:], lnsb[:])
            sk = pool.tile([NPART, len(PAIRS)], f32, name="sk")
            nc.sync.dma_start(sk[:], skt[:])
            Te = pool.tile([NPART, FIN], f16, name="Te")
            nc.sync.dma_start(Te[:], xt[:])
            small_dma = nc.gpsimd.dma_start if HEAD_OPT else nc.sync.dma_start
            ip = pool.tile([NPART, NPART], f16, name="ip")
            small_dma(ip[:], identp[:])
            im = pool.tile([NPART, NPART], f16, name="im")
            small_dma(im[:], identn[:])
            # To[i] = patch[i+1]: shifts yl by one within each patch column so
            # odd patch y-offsets become 4B-aligned reads (DVE 2x fp16).
            # Loaded straight from DRAM so it overlaps the Te DMA.
            To = pool.tile([NPART, FIN], f16, name="To")
            xta = xt[:]
            nc.scalar.dma_start(
                To[:, 0:FIN - 1],
                bass.AP(xta.tensor, xta.offset + 1, [[FIN, NPART], [1, FIN - 1]]))
            # warm the ACT spline tables (Abs/Square/Exp set) during input DMA
            warm = pool.tile([NPART, 2], f16, name="warm")
            nc.scalar.activation(warm[:], bias[:, 0:1].bitcast(f16),
                                 AFT.Exp, bias=0.0, scale=0.0)

            acc = psp.tile([NPART, 4 * F], f32, name="acc")

            def patch_ap(dy_first, x0, y0, exx, ey, nch=C, ch0=0):
                """Read [c][xl: exx][yl: ey] at patch (x0, y0); picks the
                shifted copy so the element offset is even."""
                t, yy = (Te, y0) if y0 % 2 == 0 else (To, y0 - 1)
                a = t[:]
                dims = [[FIN, NPART], [CS, nch], [PS, exx], [1, ey]]
                if nch == 1:
                    dims = [dims[0]] + dims[2:]
                return bass.AP(a.tensor, a.offset + ch0 * CS + x0 * PS + yy,
                               dims)

            def fld(t, nch, exx, ey, ch0=0, bcast=False):
                a = t[:]
                dims = [[a.shape[1], NPART], [0 if bcast else CF, nch],
                        [SX, exx], [1, ey]]
                if nch == 1:
                    dims = [dims[0]] + dims[2:]
                return bass.AP(a.tensor, a.offset + ch0 * CF, dims)

            started = [False] * 8
            npairs = min(ntaps, len(PAIRS))
            live = {}

            def stage1(k):
                dy, dx = PAIRS[k]
                EX, EY = TS + abs(dx), TS + dy
                xt0 = PAD + min(dx, 0)          # tap patch x base
                xc0 = PAD - max(dx, 0)          # center patch x base
                dlt = tp.tile([NPART, C * CF], f16, name="dlt", tag="dlt")
                if SUB2_POOL(k):
                    nc.vector.tensor_tensor(
                        out=fld(dlt, 2, EX, EY),
                        in0=patch_ap(None, xt0, PAD, EX, EY, nch=2),
                        in1=patch_ap(None, xc0, PAD - dy, EX, EY, nch=2),
                        op=AOT.subtract)
                    nc.gpsimd.tensor_tensor(
                        out=fld(dlt, 1, EX, EY, ch0=2),
                        in0=patch_ap(None, xt0, PAD, EX, EY, nch=1, ch0=2),
                        in1=patch_ap(None, xc0, PAD - dy, EX, EY, nch=1, ch0=2),
                        op=AOT.subtract)
                else:
                    nc.vector.tensor_tensor(
                        out=fld(dlt, C, EX, EY),
                        in0=patch_ap(None, xt0, PAD, EX, EY),
                        in1=patch_ap(None, xc0, PAD - dy, EX, EY),
                        op=AOT.subtract)
                # d = |dlt0|+|dlt1|+|dlt2|: abs split ACT/DVE per-pair to
                # balance engine load (DVE path: u32 sign-mask at 4 fp16/cyc)
                nact = 3 if ABS3_ACT(k) else (2 if CH1_ACT(k) else 1)
                a0 = tp.tile([NPART, 2 * CF], f16, name="a0", tag="a0")
                nc.scalar.activation(fld(a0, nact, EX, EY),
                                     fld(dlt, nact, EX, EY),
                                     AFT.Abs, bias=0.0, scale=1.0)
                adl = tp.tile([NPART, CF], f16, name="adl", tag="adl")
                if nact < 3:
                    nc.vector.tensor_scalar(
                        out=adl[:, 0:(3 - nact) * CF].bitcast(mybir.dt.uint32),
                        in0=dlt[:, nact * CF:3 * CF].bitcast(mybir.dt.uint32),
                        scalar1=0x7FFF7FFF, scalar2=None, op0=AOT.bitwise_and)
                live[k] = {"dlt": dlt, "a0": a0, "adl": adl, "nact": nact}

            def stage2a(k):
                dy, dx = PAIRS[k]
                EX, EY = TS + abs(dx), TS + dy
                t = live[k]
                nact = t["nact"]
                ch1s = t["a0"] if nact >= 2 else t["adl"]
                ch1o = 1 if nact >= 2 else 0
                s1 = tp.tile([NPART, CF], f16, name="s1", tag="s1")
                nc.gpsimd.tensor_tensor(
                    out=fld(s1, 1, EX, EY), in0=fld(t["a0"], 1, EX, EY),
                    in1=fld(ch1s, 1, EX, EY, ch0=ch1o), op=AOT.add)
                t["s1"] = s1

            def stage2b(k):
                dy, dx = PAIRS[k]
                EX, EY = TS + abs(dx), TS + dy
                t = live[k]
                nact = t["nact"]
                ch2s = t["a0"] if nact == 3 else t["adl"]
                ch2o = 2 if nact == 3 else (0 if nact == 2 else 1)
                s1 = t["s1"]
                d = tp.tile([NPART, CF], f16, name="d", tag="d")
                dadd = nc.vector if DADD_DVE(k) else nc.gpsimd
                dadd.tensor_tensor(
                    out=fld(d, 1, EX, EY), in0=fld(s1, 1, EX, EY),
                    in1=fld(ch2s, 1, EX, EY, ch0=ch2o), op=AOT.add)
                _emit_qw(k, d, EX, EY, t)

            def stage2(k):
                dy, dx = PAIRS[k]
                EX, EY = TS + abs(dx), TS + dy
                t = live[k]
                # s1 = |dlt0|+|dlt1| on GPSIMD, d = s1+|dlt2| mostly on DVE
                nact = t["nact"]
                ch1s = t["a0"] if nact >= 2 else t["adl"]
                ch1o = 1 if nact >= 2 else 0
                ch2s = t["a0"] if nact == 3 else t["adl"]
                ch2o = 2 if nact == 3 else (0 if nact == 2 else 1)
                s1 = tp.tile([NPART, CF], f16, name="s1", tag="s1")
                nc.gpsimd.tensor_tensor(
                    out=fld(s1, 1, EX, EY), in0=fld(t["a0"], 1, EX, EY),
                    in1=fld(ch1s, 1, EX, EY, ch0=ch1o), op=AOT.add)
                d = tp.tile([NPART, CF], f16, name="d", tag="d")
                dadd = nc.vector if DADD_DVE(k) else nc.gpsimd
                dadd.tensor_tensor(
                    out=fld(d, 1, EX, EY), in0=fld(s1, 1, EX, EY),
                    in1=fld(ch2s, 1, EX, EY, ch0=ch2o), op=AOT.add)
                _emit_qw(k, d, EX, EY, t)

            def _emit_qw(k, d, EX, EY, t):
                q = tp.tile([NPART, CF], f16, name="q", tag="q")
                w = tp.tile([NPART, CF], f16, name="w", tag="w")
                if GAUSS_DE(k):
                    # exp(-50 d^2) = (sqrt(pi)/2) * DerivErf(sqrt(50) d);
                    # s_k * sqrt(pi)/2 folded into a DVE 4x tensor_scalar
                    nc.scalar.activation(fld(q, 1, EX, EY), fld(d, 1, EX, EY),
                                         AFT.Derivative_Erf, bias=0.0,
                                         scale=(-SCL) ** 0.5)
                    nc.vector.tensor_scalar(
                        out=fld(w, 1, EX, EY), in0=fld(q, 1, EX, EY),
                        scalar1=sk[:, k:k + 1], scalar2=None, op0=AOT.mult)
                else:
                    if SQ_DVE(k):
                        nc.vector.tensor_tensor(
                            out=fld(q, 1, EX, EY), in0=fld(d, 1, EX, EY),
                            in1=fld(d, 1, EX, EY), op=AOT.mult)
                    else:
                        nc.scalar.activation(fld(q, 1, EX, EY),
                                             fld(d, 1, EX, EY),
                                             AFT.Square, bias=0.0, scale=1.0)
                    nc.scalar.activation(fld(w, 1, EX, EY), fld(q, 1, EX, EY),
                                         AFT.Exp, bias=bias[:, k:k + 1],
                                         scale=SCL)
                t["w"] = w

            def stage3(k):
                dy, dx = PAIRS[k]
                EX, EY = TS + abs(dx), TS + dy
                last = (k == npairs - 1)
                t = live.pop(k)
                w = t["w"]
                P = tp.tile([NPART, C * CF], f16, name="P", tag="P")
                nc.vector.tensor_tensor(
                    out=fld(P, C, EX, EY), in0=fld(t["dlt"], C, EX, EY),
                    in1=fld(w, C, EX, EY, bcast=True), op=AOT.mult)

                # PE accumulate. Pass A (+I): [P|w] at field base (max(dx,0), dy)
                # covers tap +D; pass B at base (max(-dx,0), 0) covers tap -D:
                # +I on w, -I on P.
                xa, xb = max(dx, 0), max(-dx, 0)

                def chunks(tt, x0, y0, wfield):
                    a = tt[:]
                    res = []
                    for c in range(1 if wfield else C):
                        for xh in range(2):
                            base = c * CF + (x0 + 16 * xh) * SX + y0
                            bank = 6 + xh if wfield else 2 * c + xh
                            res.append((bank, bass.AP(
                                a.tensor, a.offset + base,
                                [[a.shape[1], NPART], [SX, 16], [1, TS]])))
                    return res

                def mm(bank, mov, stat, stop):
                    nc.tensor.matmul(acc[:, bank * 512:(bank + 1) * 512],
                                     stat[:], mov,
                                     start=not started[bank], stop=stop)
                    started[bank] = True

                for bank, mov in chunks(P, xa, dy, False):
                    mm(bank, mov, ip, False)
                for bank, mov in chunks(w, xa, dy, True):
                    mm(bank, mov, ip, False)
                for bank, mov in chunks(w, xb, 0, True):
                    mm(bank, mov, ip, last)
                for bank, mov in chunks(P, xb, 0, False):
                    mm(bank, mov, im, last)

            # software pipeline: each engine's in-order queue sees work for
            # pair k+1/k+2 before the cross-engine chain of pair k resolves
            if DEPTH4:
                for kk in range(npairs + 3):
                    if kk < npairs:
                        stage1(kk)
                    if 0 <= kk - 1 < npairs:
                        stage2a(kk - 1)
                    if 0 <= kk - 2 < npairs:
                        stage2b(kk - 2)
                    if 0 <= kk - 3 < npairs:
                        stage3(kk - 3)
            else:
                for kk in range(npairs + 2):
                    if kk < npairs:
                        stage1(kk)
                    if 0 <= kk - 1 < npairs:
                        stage2(kk - 1)
                    if 0 <= kk - 2 < npairs:
                        stage3(kk - 2)

            # out = x + num * (1/(den + s0)); per-channel for ACT/DVE overlap
            den = pool.tile([NPART, F], f32, name="den")
            nc.scalar.activation(den[:], acc[:, 3 * F:], AFT.Identity,
                                 bias=bias[:, len(PAIRS):len(PAIRS) + 1],
                                 scale=1.0)
            rc = pool.tile([NPART, F], f16, name="rc")
            nsb = pool.tile([NPART, C * F], f16, name="nsb")
            t16 = pool.tile([NPART, C * F], f16, name="t16")
            o16 = pool.tile([NPART, C * F], f16, name="o16")
            def copy_c(c):
                nc.scalar.activation(nsb[:, c * F:(c + 1) * F],
                                     acc[:, c * F:(c + 1) * F], AFT.Copy,
                                     bias=0.0, scale=1.0)
            if TAIL_OPT:
                for c in range(C):
                    copy_c(c)
            with nc.allow_low_precision(reason="fp16 out within 2e-2 budget"):
                nc.vector.reciprocal(rc[:], den[:])
            rca = rc[:]
            for c in range(C):
                if not TAIL_OPT:
                    copy_c(c)
                nc.vector.tensor_tensor(
                    out=t16[:, c * F:(c + 1) * F],
                    in0=nsb[:, c * F:(c + 1) * F],
                    in1=bass.AP(rca.tensor, rca.offset, [[F, NPART], [1, F]]),
                    op=AOT.mult)
                t16a, o16a = t16[:], o16[:]
                nc.vector.tensor_tensor(
                    out=bass.AP(o16a.tensor, o16a.offset + c * F,
                                [[C * F, NPART], [TS, TS], [1, TS]]),
                    in0=bass.AP(t16a.tensor, t16a.offset + c * F,
                                [[C * F, NPART], [TS, TS], [1, TS]]),
                    in1=patch_ap(None, PAD, PAD, TS, TS, nch=1, ch0=c),
                    op=AOT.add)
                nc.sync.dma_start(o[:, c * F:(c + 1) * F],
                                  o16[:, c * F:(c + 1) * F])
    return nc


_LNSB = np.broadcast_to(
    np.concatenate([np.log(_S_PAIR), [_S0]]).astype(np.float32)[None, :],
    (NPART, len(PAIRS) + 1)).copy()
_SKT = np.broadcast_to(
    (_S_PAIR * np.sqrt(np.pi) / 2).astype(np.float32)[None, :],
    (NPART, len(PAIRS))).copy()

_NC_CACHE = {}


def _get_nc():
    if "nc" not in _NC_CACHE:
        nc = _build()
        nc.finalize()
        _NC_CACHE["nc"] = nc
    return _NC_CACHE["nc"]


def make_in_maps(x):
    xp = np.pad(x, ((0, 0), (0, 0), (PAD, PAD), (PAD, PAD)), mode="reflect")
    in_maps = []
    eye = np.eye(NPART, dtype=np.float16)
    for core in range(N_CORES):
        b, half = core // 2, core % 2
        r0 = half * ROWS
        shard = xp[b, :, r0:r0 + ROWS + 2 * PAD, :]       # (3, 262, 518)
        st = np.ascontiguousarray(shard.transpose(0, 2, 1))  # (3, 518, 262)
        se = st.strides
        v = np.lib.stride_tricks.as_strided(
            st,
            shape=(RY, GX, C, PS, PS),
            strides=(TS * se[2], TS * se[1], se[0], se[1], se[2]))
        buf = np.ascontiguousarray(v).astype(np.float16).reshape(NPART, FIN)
        in_maps.append({"xt": buf, "lnsb": _LNSB, "skt": _SKT,
                        "identp": eye, "identn": -eye})
    return in_maps


def kernel(input: np.ndarray) -> np.ndarray:
    x = np.asarray(input, dtype=np.float32)
    assert x.shape == (B, C, H, W)
    in_maps = make_in_maps(x)
    nc = _get_nc()
    res = bass_utils.run_bass_kernel_spmd(nc, in_maps, list(range(N_CORES)))
    out = np.empty((B, C, H, W), np.float32)
    for core in range(N_CORES):
        b, half = core // 2, core % 2
        r0 = half * ROWS
        ov = np.asarray(res.results[core]["o"]).astype(np.float32)
        ov = ov.reshape(RY, GX, C, TS, TS)              # (ry, gx, c, xf, yf)
        ov = ov.transpose(2, 0, 4, 1, 3).reshape(C, ROWS, W)
        out[b, :, r0:r0 + ROWS, :] = ov
    return out


# revision 3
# speedup vs baseline: 5.8638x; 3.0812x over previous
"""Bilateral blur (7x7, L1 color distance) on 8 Trainium2 NeuronCores, v2.

Input (4, 3, 512, 512) fp32 -> output (4, 3, 512, 512) fp32.

Sharding: core i handles batch i//2, row-half i%2 (256x512 px). Each of the
128 partitions owns a 32x32 output tile (16 col-groups x 8 row-slices) and
holds the matching 38x38 padded patch per channel in fp16 ([c][xl][yl],
yl contiguous), plus a 1-element-shifted copy so every tap read is 4-byte
aligned (DVE 2x fp16 mode).

Algorithm (per pair of symmetric taps +/-D, D=(dy,dx), 24 pairs + center):
  out = x + (sum_k w_k * dlt_k) / (sum_k w_k),  dlt_k = x(p+D_k) - x(p)
The weight field W(q) = s_D * exp(-50 * d(q)^2), d = sum_c |dlt_c|, is
SYMMETRIC: tap -D at pixel p uses W(p-D), and its numerator contribution is
-P(p-D) where P = W (*) dlt is the same product field used by tap +D. So
distance, exp and the multiply are computed ONCE per pair over a slightly
extended domain (EX=32+|dx|, EY=32+dy), and the PE accumulates both taps
via identity matmuls into PSUM: +I on [P|W](p-domain), -I on P and +I on W
at the mirrored offset. The center tap is exact: contributes only s0 to the
denominator (folded into the PSUM->SBUF copy as an ACT bias).

Engine split per pair (balanced ~4.7us/pair across engines): DVE does the
subtract, |dlt2| (u32 sign-mask at 4 fp16/cyc), the second channel-sum add,
and the product (fp16 TT 2x); ACT does |dlt0|,|dlt1| (one 2-channel Abs),
Square and Exp (s_k rides the exp bias as ln s_k); Pool (GPSIMD) does the
first channel-sum add; PE does 16 accumulate matmuls. Emission is
software-pipelined in 3 stages so each in-order engine queue always has
ready work. ACT tables are pre-warmed during the input DMA.
"""
import numpy as np

import concourse.bass as bass
import concourse.bacc as bacc
import concourse.mybir as mybir
from concourse.tile import TileContext
from concourse import bass_utils

C = 3
B, H, W = 4, 512, 512
PAD = 3
SIGMA_COLOR = 0.1
N_CORES = 8

TS = 32                      # tile side (output px per partition: TS x TS)
PS = TS + 2 * PAD            # padded patch side = 38
NPART = 128
GX, RY = 16, 8               # col-groups x row-slices = 128 partitions
ROWS = RY * TS               # 256 output rows per core
CS = PS * PS                 # per-channel patch stride = 1444
FIN = C * CS                 # 4332
SX = 36                      # field xl stride (even, >= max EY=35)
CF = SX * 35                 # per-channel field stride = 1260 (even)
F = TS * TS                  # 1024
SCL = -0.5 / SIGMA_COLOR ** 2

# symmetric tap pairs: (dy, dx) with dy>0, or dy==0 and dx>0
_P0 = [(0, dx) for dx in range(1, 4)] + \
      [(dy, dx) for dy in range(1, 4) for dx in range(-3, 4)]
# small fields at pipeline fill/drain ends, big in the middle
_PS = sorted(_P0, key=lambda p: (32 + abs(p[1])) * (32 + p[0]))
PAIRS = _PS[:12][0::2] + _PS[12:] + _PS[:12][1::2][::-1]
# per-pair engine assignment knobs (k -> bool), tuned via TimelineSim sweeps
CH1_ACT = lambda k: True      # |dlt1| on ACT (with ch0) vs DVE u32
DADD_DVE = lambda k: True     # d = s1+|dlt2| on DVE vs GPSIMD
SUB2_POOL = lambda k: False   # ch2 of the subtract on GPSIMD (parallel lane)
SQ_DVE = lambda k: False      # square on DVE (d*d) vs ACT
ABS3_ACT = lambda k: False    # all 3 abs channels on ACT (no DVE u32 op)
GAUSS_DE = lambda k: False    # Derivative_Erf gaussian + DVE 4x s_k scale
HEAD_OPT = True               # small-table DMAs off the sync queue
DEPTH4 = False                # 4-deep software pipeline (split stage2)
TAIL_OPT = True               # PSUM->SBUF copies before the reciprocal


def _g1(k, sigma):
    x = np.arange(k, dtype=np.float64) - (k - 1) / 2.0
    g = np.exp(-0.5 * (x / sigma) ** 2)
    return g / g.sum()


_G = _g1(7, 1.5)
_S_PAIR = np.array([_G[3 + dy] * _G[3 + dx] for (dy, dx) in PAIRS])
_S0 = float(_G[3] * _G[3])


def _build(ntaps=len(PAIRS)):
    nc = bacc.Bacc()
    f32 = mybir.dt.float32
    f16 = mybir.dt.float16
    xt = nc.dram_tensor("xt", [NPART, FIN], f16, kind="ExternalInput")
    lnsb = nc.dram_tensor("lnsb", [NPART, len(PAIRS) + 1], f32,
                          kind="ExternalInput")
    skt = nc.dram_tensor("skt", [NPART, len(PAIRS)], f32,
                         kind="ExternalInput")
    identp = nc.dram_tensor("identp", [NPART, NPART], f16, kind="ExternalInput")
    identn = nc.dram_tensor("identn", [NPART, NPART], f16, kind="ExternalInput")
    o = nc.dram_tensor("o", [NPART, C * F], f16, kind="ExternalOutput")
    AOT = mybir.AluOpType
    AFT = mybir.ActivationFunctionType

    with TileContext(nc) as tc:
        with tc.tile_pool(name="persist", bufs=1) as pool, \
             tc.tile_pool(name="tmpb", bufs=5) as tpb, \
             tc.tile_pool(name="tmp", bufs=4) as tp, \
             tc.tile_pool(name="ps", bufs=1, space="PSUM") as psp:
            bias = pool.tile([NPART, len(PAIRS) + 1], f32, name="bias")
            nc.sync.dma_start(bias[:], lnsb[:])
            sk = pool.tile([NPART, len(PAIRS)], f32, name="sk")
            nc.sync.dma_start(sk[:], skt[:])
            Te = pool.tile([NPART, FIN], f16, name="Te")
            nc.sync.dma_start(Te[:], xt[:])
            small_dma = nc.gpsimd.dma_start if HEAD_OPT else nc.sync.dma_start
            ip = pool.tile([NPART, NPART], f16, name="ip")
            small_dma(ip[:], identp[:])
            im = pool.tile([NPART, NPART], f16, name="im")
            small_dma(im[:], identn[:])
            # To[i] = patch[i+1]: shifts yl by one within each patch column so
            # odd patch y-offsets become 4B-aligned reads (DVE 2x fp16).
            # Loaded straight from DRAM so it overlaps the Te DMA.
            To = pool.tile([NPART, FIN], f16, name="To")
            xta = xt[:]
            nc.scalar.dma_start(
                To[:, 0:FIN - 1],
                bass.AP(xta.tensor, xta.offset + 1, [[FIN, NPART], [1, FIN - 1]]))
            # warm the ACT spline tables (Abs/Square/Exp set) during input DMA
            warm = pool.tile([NPART, 2], f16, name="warm")
            nc.scalar.activation(warm[:], bias[:, 0:1].bitcast(f16),
                                 AFT.Exp, bias=0.0, scale=0.0)

            acc = psp.tile([NPART, 4 * F], f32, name="acc")

            def patch_ap(dy_first, x0, y0, exx, ey, nch=C, ch0=0):
                """Read [c][xl: exx][yl: ey] at patch (x0, y0); picks the
                shifted copy so the element offset is even."""
                t, yy = (Te, y0) if y0 % 2 == 0 else (To, y0 - 1)
                a = t[:]
                dims = [[FIN, NPART], [CS, nch], [PS, exx], [1, ey]]
                if nch == 1:
                    dims = [dims[0]] + dims[2:]
                return bass.AP(a.tensor, a.offset + ch0 * CS + x0 * PS + yy,
                               dims)

            def fld(t, nch, exx, ey, ch0=0, bcast=False):
                a = t[:]
                dims = [[a.shape[1], NPART], [0 if bcast else CF, nch],
                        [SX, exx], [1, ey]]
                if nch == 1:
                    dims = [dims[0]] + dims[2:]
                return bass.AP(a.tensor, a.offset + ch0 * CF, dims)

            started = [False] * 8
            npairs = min(ntaps, len(PAIRS))
            live = {}

            def stage1(k):
                dy, dx = PAIRS[k]
                EX, EY = TS + abs(dx), TS + dy
                xt0 = PAD + min(dx, 0)          # tap patch x base
                xc0 = PAD - max(dx, 0)          # center patch x base
                dlt = tpb.tile([NPART, C * CF], f16, name="dlt", tag="dlt")
                if SUB2_POOL(k):
                    nc.vector.tensor_tensor(
                        out=fld(dlt, 2, EX, EY),
                        in0=patch_ap(None, xt0, PAD, EX, EY, nch=2),
                        in1=patch_ap(None, xc0, PAD - dy, EX, EY, nch=2),
                        op=AOT.subtract)
                    nc.gpsimd.tensor_tensor(
                        out=fld(dlt, 1, EX, EY, ch0=2),
                        in0=patch_ap(None, xt0, PAD, EX, EY, nch=1, ch0=2),
                        in1=patch_ap(None, xc0, PAD - dy, EX, EY, nch=1, ch0=2),
                        op=AOT.subtract)
                else:
                    nc.vector.tensor_tensor(
                        out=fld(dlt, C, EX, EY),
                        in0=patch_ap(None, xt0, PAD, EX, EY),
                        in1=patch_ap(None, xc0, PAD - dy, EX, EY),
                        op=AOT.subtract)
                # d = |dlt0|+|dlt1|+|dlt2|: abs split ACT/DVE per-pair to
                # balance engine load (DVE path: u32 sign-mask at 4 fp16/cyc)
                nact = 3 if ABS3_ACT(k) else (2 if CH1_ACT(k) else 1)
                a0 = tp.tile([NPART, 2 * CF], f16, name="a0", tag="a0")
                nc.scalar.activation(fld(a0, nact, EX, EY),
                                     fld(dlt, nact, EX, EY),
                                     AFT.Abs, bias=0.0, scale=1.0)
                adl = tp.tile([NPART, CF], f16, name="adl", tag="adl")
                if nact < 3:
                    nc.vector.tensor_scalar(
                        out=adl[:, 0:(3 - nact) * CF].bitcast(mybir.dt.uint32),
                        in0=dlt[:, nact * CF:3 * CF].bitcast(mybir.dt.uint32),
                        scalar1=0x7FFF7FFF, scalar2=None, op0=AOT.bitwise_and)
                live[k] = {"dlt": dlt, "a0": a0, "adl": adl, "nact": nact}

            def stage2a(k):
                dy, dx = PAIRS[k]
                EX, EY = TS + abs(dx), TS + dy
                t = live[k]
                nact = t["nact"]
                ch1s = t["a0"] if nact >= 2 else t["adl"]
                ch1o = 1 if nact >= 2 else 0
                s1 = tp.tile([NPART, CF], f16, name="s1", tag="s1")
                nc.gpsimd.tensor_tensor(
                    out=fld(s1, 1, EX, EY), in0=fld(t["a0"], 1, EX, EY),
                    in1=fld(ch1s, 1, EX, EY, ch0=ch1o), op=AOT.add)
                t["s1"] = s1

            def stage2b(k):
                dy, dx = PAIRS[k]
                EX, EY = TS + abs(dx), TS + dy
                t = live[k]
                nact = t["nact"]
                ch2s = t["a0"] if nact == 3 else t["adl"]
                ch2o = 2 if nact == 3 else (0 if nact == 2 else 1)
                s1 = t["s1"]
                d = tp.tile([NPART, CF], f16, name="d", tag="d")
                dadd = nc.vector if DADD_DVE(k) else nc.gpsimd
                dadd.tensor_tensor(
                    out=fld(d, 1, EX, EY), in0=fld(s1, 1, EX, EY),
                    in1=fld(ch2s, 1, EX, EY, ch0=ch2o), op=AOT.add)
                _emit_qw(k, d, EX, EY, t)

            def stage2(k):
                dy, dx = PAIRS[k]
                EX, EY = TS + abs(dx), TS + dy
                t = live[k]
                # s1 = |dlt0|+|dlt1| on GPSIMD, d = s1+|dlt2| mostly on DVE
                nact = t["nact"]
                ch1s = t["a0"] if nact >= 2 else t["adl"]
                ch1o = 1 if nact >= 2 else 0
                ch2s = t["a0"] if nact == 3 else t["adl"]
                ch2o = 2 if nact == 3 else (0 if nact == 2 else 1)
                s1 = tp.tile([NPART, CF], f16, name="s1", tag="s1")
                nc.gpsimd.tensor_tensor(
                    out=fld(s1, 1, EX, EY), in0=fld(t["a0"], 1, EX, EY),
                    in1=fld(ch1s, 1, EX, EY, ch0=ch1o), op=AOT.add)
                d = tp.tile([NPART, CF], f16, name="d", tag="d")
                dadd = nc.vector if DADD_DVE(k) else nc.gpsimd
                dadd.tensor_tensor(
                    out=fld(d, 1, EX, EY), in0=fld(s1, 1, EX, EY),
                    in1=fld(ch2s, 1, EX, EY, ch0=ch2o), op=AOT.add)
                _emit_qw(k, d, EX, EY, t)

            def _emit_qw(k, d, EX, EY, t):
                q = tp.tile([NPART, CF], f16, name="q", tag="q")
                w = tp.tile([NPART, CF], f16, name="w", tag="w")
                if GAUSS_DE(k):
                    # exp(-50 d^2) = (sqrt(pi)/2) * DerivErf(sqrt(50) d);
                    # s_k * sqrt(pi)/2 folded into a DVE 4x tensor_scalar
                    nc.scalar.activation(fld(q, 1, EX, EY), fld(d, 1, EX, EY),
                                         AFT.Derivative_Erf, bias=0.0,
                                         scale=(-SCL) ** 0.5)
                    nc.vector.tensor_scalar(
                        out=fld(w, 1, EX, EY), in0=fld(q, 1, EX, EY),
                        scalar1=sk[:, k:k + 1], scalar2=None, op0=AOT.mult)
                else:
                    if SQ_DVE(k):
                        nc.vector.tensor_tensor(
                            out=fld(q, 1, EX, EY), in0=fld(d, 1, EX, EY),
                            in1=fld(d, 1, EX, EY), op=AOT.mult)
                    else:
                        nc.scalar.activation(fld(q, 1, EX, EY),
                                             fld(d, 1, EX, EY),
                                             AFT.Square, bias=0.0, scale=1.0)
                    nc.scalar.activation(fld(w, 1, EX, EY), fld(q, 1, EX, EY),
                                         AFT.Exp, bias=bias[:, k:k + 1],
                                         scale=SCL)
                t["w"] = w

            def stage3(k):
                dy, dx = PAIRS[k]
                EX, EY = TS + abs(dx), TS + dy
                last = (k == npairs - 1)
                t = live.pop(k)
                w = t["w"]
                P = tpb.tile([NPART, C * CF], f16, name="P", tag="P")
                nc.vector.tensor_tensor(
                    out=fld(P, C, EX, EY), in0=fld(t["dlt"], C, EX, EY),
                    in1=fld(w, C, EX, EY, bcast=True), op=AOT.mult)

                # PE accumulate. Pass A (+I): [P|w] at field base (max(dx,0), dy)
                # covers tap +D; pass B at base (max(-dx,0), 0) covers tap -D:
                # +I on w, -I on P.
                xa, xb = max(dx, 0), max(-dx, 0)

                def chunks(tt, x0, y0, wfield):
                    a = tt[:]
                    res = []
                    for c in range(1 if wfield else C):
                        for xh in range(2):
                            base = c * CF + (x0 + 16 * xh) * SX + y0
                            bank = 6 + xh if wfield else 2 * c + xh
                            res.append((bank, bass.AP(
                                a.tensor, a.offset + base,
                                [[a.shape[1], NPART], [SX, 16], [1, TS]])))
                    return res

                def mm(bank, mov, stat, stop):
                    nc.tensor.matmul(acc[:, bank * 512:(bank + 1) * 512],
                                     stat[:], mov,
                                     start=not started[bank], stop=stop)
                    started[bank] = True

                for bank, mov in chunks(P, xa, dy, False):
                    mm(bank, mov, ip, False)
                for bank, mov in chunks(w, xa, dy, True):
                    mm(bank, mov, ip, False)
                for bank, mov in chunks(w, xb, 0, True):
                    mm(bank, mov, ip, last)
                for bank, mov in chunks(P, xb, 0, False):
                    mm(bank, mov, im, last)

            # software pipeline: each engine's in-order queue sees work for
            # pair k+1/k+2 before the cross-engine chain of pair k resolves
            if DEPTH4:
                for kk in range(npairs + 3):
                    if kk < npairs:
                        stage1(kk)
                    if 0 <= kk - 1 < npairs:
                        stage2a(kk - 1)
                    if 0 <= kk - 2 < npairs:
                        stage2b(kk - 2)
                    if 0 <= kk - 3 < npairs:
                        stage3(kk - 3)
            else:
                for kk in range(npairs + 2):
                    if kk < npairs:
                        stage1(kk)
                    if 0 <= kk - 1 < npairs:
                        stage2(kk - 1)
                    if 0 <= kk - 2 < npairs:
                        stage3(kk - 2)

            # out = x + num * (1/(den + s0)); per-channel for ACT/DVE overlap
            den = pool.tile([NPART, F], f32, name="den")
            nc.scalar.activation(den[:], acc[:, 3 * F:], AFT.Identity,
                                 bias=bias[:, len(PAIRS):len(PAIRS) + 1],
                                 scale=1.0)
            rc = pool.tile([NPART, F], f16, name="rc")
            nsb = pool.tile([NPART, C * F], f16, name="nsb")
            t16 = pool.tile([NPART, C * F], f16, name="t16")
            o16 = pool.tile([NPART, C * F], f16, name="o16")
            def copy_c(c):
                nc.scalar.activation(nsb[:, c * F:(c + 1) * F],
                                     acc[:, c * F:(c + 1) * F], AFT.Copy,
                                     bias=0.0, scale=1.0)
            if TAIL_OPT:
                for c in range(C):
                    copy_c(c)
            with nc.allow_low_precision(reason="fp16 out within 2e-2 budget"):
                nc.vector.reciprocal(rc[:], den[:])
            rca = rc[:]
            for c in range(C):
                if not TAIL_OPT:
                    copy_c(c)
                nc.vector.tensor_tensor(
                    out=t16[:, c * F:(c + 1) * F],
                    in0=nsb[:, c * F:(c + 1) * F],
                    in1=bass.AP(rca.tensor, rca.offset, [[F, NPART], [1, F]]),
                    op=AOT.mult)
                t16a, o16a = t16[:], o16[:]
                nc.vector.tensor_tensor(
                    out=bass.AP(o16a.tensor, o16a.offset + c * F,
                                [[C * F, NPART], [TS, TS], [1, TS]]),
                    in0=bass.AP(t16a.tensor, t16a.offset + c * F,
                                [[C * F, NPART], [TS, TS], [1, TS]]),
                    in1=patch_ap(None, PAD, PAD, TS, TS, nch=1, ch0=c),
                    op=AOT.add)
                nc.sync.dma_start(o[:, c * F:(c + 1) * F],
                                  o16[:, c * F:(c + 1) * F])
    return nc


_LNSB = np.broadcast_to(
    np.concatenate([np.log(_S_PAIR), [_S0]]).astype(np.float32)[None, :],
    (NPART, len(PAIRS) + 1)).copy()
_SKT = np.broadcast_to(
    (_S_PAIR * np.sqrt(np.pi) / 2).astype(np.float32)[None, :],
    (NPART, len(PAIRS))).copy()

_NC_CACHE = {}


def _get_nc():
    if "nc" not in _NC_CACHE:
        nc = _build()
        nc.finalize()
        _NC_CACHE["nc"] = nc
    return _NC_CACHE["nc"]


def make_in_maps(x):
    xp = np.pad(x, ((0, 0), (0, 0), (PAD, PAD), (PAD, PAD)), mode="reflect")
    in_maps = []
    eye = np.eye(NPART, dtype=np.float16)
    for core in range(N_CORES):
        b, half = core // 2, core % 2
        r0 = half * ROWS
        shard = xp[b, :, r0:r0 + ROWS + 2 * PAD, :]       # (3, 262, 518)
        st = np.ascontiguousarray(shard.transpose(0, 2, 1))  # (3, 518, 262)
        se = st.strides
        v = np.lib.stride_tricks.as_strided(
            st,
            shape=(RY, GX, C, PS, PS),
            strides=(TS * se[2], TS * se[1], se[0], se[1], se[2]))
        buf = np.ascontiguousarray(v).astype(np.float16).reshape(NPART, FIN)
        in_maps.append({"xt": buf, "lnsb": _LNSB, "skt": _SKT,
                        "identp": eye, "identn": -eye})
    return in_maps


def kernel(input: np.ndarray) -> np.ndarray:
    x = np.asarray(input, dtype=np.float32)
    assert x.shape == (B, C, H, W)
    in_maps = make_in_maps(x)
    nc = _get_nc()
    res = bass_utils.run_bass_kernel_spmd(nc, in_maps, list(range(N_CORES)))
    out = np.empty((B, C, H, W), np.float32)
    for core in range(N_CORES):
        b, half = core // 2, core % 2
        r0 = half * ROWS
        ov = np.asarray(res.results[core]["o"]).astype(np.float32)
        ov = ov.reshape(RY, GX, C, TS, TS)              # (ry, gx, c, xf, yf)
        ov = ov.transpose(2, 0, 4, 1, 3).reshape(C, ROWS, W)
        out[b, :, r0:r0 + ROWS, :] = ov
    return out
